# revision 92
# baseline (speedup 1.0000x reference)
"""Distributed inverse real SHT on 8 Trainium2 NeuronCores (Bass/Tile).

Math (per reference):
    S[c,k,m]  = sum_l x[c,m,l] * pct[m,k,l]          (Legendre synthesis)
    y[c,k,n]  = irfft_{n=1024}(S, norm='forward')
              = sum_m  Sre[c,k,m]*Fc[m,n] + Sim[c,k,m]*Fs[m,n]
    with Fc[m,n] = w_m cos(2*pi*m*n/N), Fs[m,n] = -w_m sin(2*pi*m*n/N),
    w_0 = 1, w_m = 2 otherwise (verified exactly vs np.fft.irfft).
    pct[m,*,l] = 0 for l < m (triangular), and the m=512 row of pct is
    entirely zero (l < 512 always), so the effective mmax is 512.

Sharding: nlat (k) split across the 8 cores -> 64 output latitudes per
core, no inter-core communication.

v6 addenda:
  - K_A2 (mirror split of the A-range at row 32 for K<=32 columns)
    measured +0.4us vs a back-to-back control despite saving the same
    0.39MB as K_B3: its extra issues land on the gpsimd queue (which
    paces slab readiness) and its split slabs sit in the startup
    region.  Left available but off by default.

v6 vs v5:
  - third DMA row-range: columns with K in (64,96] skip rows 96:128
    (0.39MB less stream, only 4 slabs split so issue overhead stays
    negligible; -0.6us vs a contemporaneous control).
  - slab ring depth stays 6 (8 measured +7us vs control: SBUF pressure,
    no slab-wait payoff).

v5 vs v4:
  - each slab's rows-64:128 range is issued from the sync queue while
    rows-0:64 stay on gpsimd: two queues feed the DMA engines
    concurrently (~1.5us).  NOTE: whole-slab queue alternation and
    per-slab contiguous DRAM tensors both HURT; only this row-range
    split helps.
  - A/B runs are only comparable within a machine-state window: the
    device band drifted +11us mid-session (thermal/co-tenant).  Re-test
    any surprising regression against a fresh same-window baseline.

v4 vs v3:
  - trailing 4 slabs are 1-bank (finer arrival granularity where the
    stage-1 tail chases the last slabs; 8 trailing 1-bank slabs or
    1-bank leading slabs measured no better).
  - odd tail strips take their E/O PSUM banks from the stage-1 ps1
    ring (same tag, same tile size, ring is idle by then): alternating
    PSUM pools doubles the WAR distance per pool in the tail.
  Additional variants measured and REJECTED: per-slab contiguous DRAM
  tensors (+11us: the DMA engines parallelize better over the
  monolithic strided layout), drain repacing 4-early/0-late (+4us) and
  2-per-bank (+4us), uv/ysb rings of 4 (+4us with K_PREF=10).

v3 vs v2:
  - x is streamed as fp8e3 (e3m4) too, not just pct: the Legendre
    matmul runs fp8e3 x fp8e3.  Cuts stream DMA from ~17.6MB to
    ~13.2MB per core.  Max-rel error 1.92e-2 (gate 2e-2, deterministic
    inputs, verified offline and on hardware).
  - Slab plan generalized: banks-per-slab list + lookahead prefetch
    are tunable (K_SLABS / K_PREF); measured optimum stays at uniform
    2-bank slabs with ~8-bank lookahead.
  Scheduling variants measured and REJECTED (each +3..18us):
  equal-byte 1.5MB slabs (consumers wait whole-slab -> coarse
  pipeline), 4-bank slabs for thin blocks, alternating slab issue
  queues, 3-block PSUM DFT grouping, PE id-seed of block1's group,
  folds reading PSUM directly (tail is vector-bound; GPS cannot read
  PSUM, TT ops allow only one PSUM input), tail thunk interleaving.

v2 vs v1:
  - pct streamed as fp8e3 (e3m4) instead of fp16.  Verified offline
    against the reference: max-rel error 1.34e-2 (gate 2e-2).
  - Slab columns sorted K-descending and DMA'd as two row-ranges
    (rows [64:128) only for columns with K > 64), removing most of the
    20% DMA row padding of v1.  (Nonzero-tile_position shelf packing
    hangs this hardware, so row-skip DMA is the packing mechanism.)
  - DFT restructured: blocks (3,2) accumulate in one PSUM group (one
    SBUF copy, no add), block 1 adds, block 0 folds directly from
    PSUM+acc in the tail.  Halves the DVE accumulate traffic; the fp32
    "acc" buffer is now fp16.
  - Slab DMAs issued from the GpSimd queue and y writeback from the
    Scalar queue to unload the saturated Sync engine.
"""

import os
import numpy as np
from contextlib import ExitStack


NLAT, NLON = 512, 1024
LMAX, MMAX = 512, 513
M_E = 512            # effective mmax (m=512 row of pct is identically zero)
B, C = 1, 16
NCORES = 8
KC = NLAT // NCORES  # 64 latitudes per core
PAIRS = M_E // 2     # 256 m-pairs
NBLK = 4             # 128-m blocks
NBANK = PAIRS // 8   # 32 PSUM banks (8 pairs each)

PCT_FP8 = os.environ.get("K_FP8", "1") == "1"  # pct as fp8e3, 2/fp16-slot
X_FP8 = os.environ.get("K_XFP8", "1") == "1"   # x as fp8e3 too
ALT_DMAQ = os.environ.get("K_DMAQ", "1") == "1"  # slab/y DMA off sync eng
B_ON_SYNC = os.environ.get("K_BSYNC", "1") == "1"  # slab rows 64:128 on sync
SP_BUFS = int(os.environ.get("K_SPBUFS", "6"))     # slab ring depth
B_3RANGE = os.environ.get("K_B3", "1") == "1"      # skip rows 96:128 K<=96
A_2RANGE = os.environ.get("K_A2", "0") == "1"      # skip rows 32:64 K<=32
UV_BUFS = int(os.environ.get("K_UVB", "3"))        # tail uv/ysb ring depth
# (ring depth 8 measured +7us against a contemporaneous control:
# deeper prefetch adds SBUF pressure with no slab-wait payoff)
PCT_W16 = 64 if PCT_FP8 else 128   # fp16 slots for the pct part of a tile
X_W16 = 32 if X_FP8 else 64        # fp16 slots for the x part (2m*2ri*16c)
TILE_W = PCT_W16 + X_W16

# processing order: shortest pairs first; each block's deferred
# transpose+DFT work hides inside the next block's DMA window
BORDER = [3, 2, 1, 0]
BANK_SEQ = [b * 8 + g for b in BORDER for g in range(8)]


PREF_BANKS = int(os.environ.get("K_PREF", "8"))  # bank lookahead
DRAIN_N = int(os.environ.get("K_DRAIN", "3"))    # deferred drains per bank
DRAIN_CUT = int(os.environ.get("K_DRAINCUT", "8"))  # no drains from here
# banks per slab along the processing order (block3, block2, block1, block0)
SLAB_BANKS = [int(c) for c in os.environ.get(
    "K_SLABS", ",".join(["2"] * 14 + ["1"] * 4)).split(",")]
assert sum(SLAB_BANKS) == NBANK


def _plan():
    """One 128-row column per (pair, l-chunk) tile.  Columns are grouped
    into slabs of ~equal DMA bytes (not equal bank counts): thin-column
    blocks pack more columns per slab, which keeps the per-partition DMA
    packet size large (~w*192B) and the DMA cadence uniform.  Within
    each slab columns are sorted K-descending so the slab DMA can skip
    rows [64:128) for the trailing columns with K <= 64.

    Returns (bank_ops, slab_col0, slab_w64, bank_slab, ncols):
    bank_ops[G] = [(t, l0, K, col)] K-descending (first op start=True);
    slab_w64[si] = column count needing rows >= 64; bank_slab[p] = max
    slab index used by the bank at processing position p."""
    # columns in processing order, with bank position attached
    cols = []
    for pos, G in enumerate(BANK_SEQ):
        for t in range(8 * G, 8 * G + 8):
            l0 = 2 * t
            L = LMAX - l0
            nch = (L + 127) // 128
            for c in range(nch):
                cols.append((pos, G, t, l0 + 128 * c,
                             min(128, L - 128 * c)))
    # slab boundaries aligned to bank-processing positions: fine slabs
    # where consumption is fast, 4-bank slabs for the thin-column blocks
    # (3, 2) so the per-partition DMA packet stays large
    pos_bounds = []
    p = 0
    for nb in SLAB_BANKS:
        p += nb
        pos_bounds.append(p)
    bounds = []
    bi_ = 0
    for i, (pos, G, t, l0, K) in enumerate(cols):
        if pos >= pos_bounds[bi_]:
            bounds.append(i)
            bi_ += 1
    bounds.append(len(cols))
    bank_ops = [[] for _ in range(NBANK)]
    slab_col0 = [0]
    slab_w64 = []
    slab_w96 = []
    slab_w32 = []
    bank_slab = [0] * NBANK
    start = 0
    for si, end in enumerate(bounds):
        tiles = sorted(cols[start:end], key=lambda x: -x[4])
        slab_w64.append(sum(1 for x in tiles if x[4] > 64))
        slab_w96.append(sum(1 for x in tiles if x[4] > 96))
        slab_w32.append(sum(1 for x in tiles if x[4] > 32))
        for i, (pos, G, t, l0, K) in enumerate(tiles):
            bank_ops[G].append((t, l0, K, slab_col0[-1] + i))
            bank_slab[pos] = max(bank_slab[pos], si)
        slab_col0.append(slab_col0[-1] + len(tiles))
        start = end
    import bisect

    def slab_of(col):
        return bisect.bisect_right(slab_col0, col) - 1

    for G in range(NBANK):
        # earlier-slab ops first so available work runs before any stall
        bank_ops[G].sort(key=lambda x: (slab_of(x[3]), -x[2]))
    return (bank_ops, slab_col0, slab_w64, slab_w96, slab_w32,
            bank_slab, len(cols))


(_BANK_OPS, _SLAB_COL0, _SLAB_W64, _SLAB_W96, _SLAB_W32, _BANK_SLAB,
 NCOLS) = _plan()
NSLABS = len(_SLAB_W64)
F_TOT = NCOLS * TILE_W


def _slab_of(col):
    import bisect
    return bisect.bisect_right(_SLAB_COL0, col) - 1

# Even/odd DFT folding: E[n'] = sum_m wc*Re and O[n'] = sum_m ws*Im
# for n' in [0,512) plus the y[512] column (folded into O's zero column);
# then y[n'] = E+O, y[1024-n'] = E-O.
NE = NLON // 2       # 512
FW = NE + NE + 16    # wc | ws | (y512 col + pad)
CKC = C * KC         # 1024


def build_program():
    from concourse import bacc, bass, masks, mybir, tile

    dt = mybir.dt
    nc = bacc.Bacc("TRN2", target_bir_lowering=False, debug=False,
                   num_devices=NCORES)

    # NOTE: per-slab contiguous DRAM tensors were measured 11us SLOWER
    # than this monolithic strided layout (the engines parallelize
    # better over uniform big-stride partition rows)
    stream = nc.dram_tensor("stream", [128, F_TOT], dt.float16,
                            kind="ExternalInput")
    fmat = nc.dram_tensor("fmat", [128, NBLK * FW + 128], dt.float16,
                          kind="ExternalInput")
    y = nc.dram_tensor("y", [CKC, NLON], dt.float16, kind="ExternalOutput")

    with tile.TileContext(nc) as tc, ExitStack() as ctx:
        sp = ctx.enter_context(
            tc.tile_pool(name="stream", bufs=SP_BUFS))
        cp = ctx.enter_context(tc.tile_pool(name="const", bufs=1))
        fp = ctx.enter_context(tc.tile_pool(name="fsb", bufs=4))
        ysp = ctx.enter_context(tc.tile_pool(name="ysb", bufs=UV_BUFS))
        uvp = ctx.enter_context(tc.tile_pool(name="uv", bufs=UV_BUFS))
        snp = ctx.enter_context(tc.tile_pool(name="snat", bufs=2))
        lhp = ctx.enter_context(tc.tile_pool(name="lhs", bufs=2))
        ps1 = ctx.enter_context(
            tc.tile_pool(name="ps1", bufs=3, space=bass.MemorySpace.PSUM))
        pst = ctx.enter_context(
            tc.tile_pool(name="pst", bufs=1, space=bass.MemorySpace.PSUM))
        ps2 = ctx.enter_context(
            tc.tile_pool(name="ps2", bufs=2, space=bass.MemorySpace.PSUM))

        # fp16 partial accumulator for the blocks handled before the tail:
        # partition = (c2,k64) within strip, free = strip*1024 + n
        acc = cp.tile([128, 8 * NLON], dt.float16)

        slabs = {}

        def get_slab(si, eng=None):
            si = min(si, NSLABS - 1)
            if si not in slabs:
                w = _SLAB_COL0[si + 1] - _SLAB_COL0[si]
                w64 = _SLAB_W64[si]
                st = sp.tile([128, w * TILE_W], dt.float16, tag="slab")
                o0 = _SLAB_COL0[si] * TILE_W
                if eng is None:
                    eng = nc.gpsimd if ALT_DMAQ else nc.sync
                # rows 64:128 go through the sync queue so two queues
                # feed the DMA engines concurrently (K_BSYNC=0 reverts)
                eng_b = nc.sync if B_ON_SYNC else eng
                w32 = _SLAB_W32[si]
                if A_2RANGE and w32 < w:
                    # columns with K <= 32 skip rows 32:64; the kept
                    # 32:64 piece rides sync so the split slab's pieces
                    # land in parallel across queues
                    eng.dma_start(
                        out=st[0:32, :],
                        in_=stream[0:32, o0:o0 + w * TILE_W])
                    if w32:
                        eng_b.dma_start(
                            out=st[32:64, 0:w32 * TILE_W],
                            in_=stream[32:64, o0:o0 + w32 * TILE_W])
                else:
                    eng.dma_start(
                        out=st[0:64, :],
                        in_=stream[0:64, o0:o0 + w * TILE_W])
                w96 = _SLAB_W96[si]
                if B_3RANGE and w64 and w96 < w64:
                    # columns with K in (64,96] skip rows 96:128 too
                    eng_b.dma_start(
                        out=st[64:96, 0:w64 * TILE_W],
                        in_=stream[64:96, o0:o0 + w64 * TILE_W])
                    if w96:
                        eng_b.dma_start(
                            out=st[96:128, 0:w96 * TILE_W],
                            in_=stream[96:128, o0:o0 + w96 * TILE_W])
                elif w64:
                    eng_b.dma_start(
                        out=st[64:128, 0:w64 * TILE_W],
                        in_=stream[64:128, o0:o0 + w64 * TILE_W])
                slabs[si] = st
            return slabs[si]

        # first slabs from the sync queue (ready before gpsimd's table
        # load); identity comes in with fmat instead of being computed
        # (slab 1 on gpsimd instead measured +5us-or-drift; reverted)
        get_slab(0, nc.sync)
        get_slab(1, nc.sync)
        get_slab(2, nc.sync)

        ident = cp.tile([128, 128], dt.float16)
        nc.sync.dma_start(out=ident[:],
                          in_=fmat[:, NBLK * FW:NBLK * FW + 128])

        deferred = []  # previous block's transpose + DFT work, as thunks

        def emit_dft(tE, oE, tO, oO, lh, fsb, s8, first, last):
            """One block's 3 DFT matmuls for strip s8 into the E bank
            (tile tE cols oE:oE+NE) and O bank (tO cols oO:oO+NE, which
            also takes the y512 column).  Each bank is one accumulation
            group."""
            l0 = lh[:, s8 * 128:s8 * 128 + 128]               # Re rows
            l1 = lh[:, CKC + s8 * 128:CKC + s8 * 128 + 128]   # Im rows
            nc.tensor.matmul(tE[:, oE:oE + NE], l0, fsb[:, 0:NE],
                             start=first, stop=last)
            nc.tensor.matmul(tO[:, oO:oO + NE], l1, fsb[:, NE:2 * NE],
                             start=first, stop=False)
            nc.tensor.matmul(tO[:, oO:oO + 8], l0,
                             fsb[:, 2 * NE:2 * NE + 8],
                             start=False, stop=last)

        def make_deferred(bi, snat_m, fsb, prev):
            """Block's post-stage-1 work: PE transposes into the stage-2
            lhsT layout, then (depending on position in the schedule)
            the DFT matmuls + accumulation.  prev = list of (lhs, fsb)
            for blocks whose DFT is still pending."""
            thunks = []
            lhs = lhp.tile([128, 2 * CKC], dt.float16, tag="lhs")
            lhs_v = lhs[:].rearrange("p (r c k) -> p r c k", r=2, c=C, k=KC)

            def transp_group(g4):
                # 4 channels' PE transposes into one PSUM tile, then a
                # single strided copy into the lhs layout
                pt4 = pst.tile([128, 512], dt.float16, tag="pt")
                for j in range(4):
                    cc = 4 * g4 + j
                    nc.tensor.transpose(pt4[:, 128 * j:128 * (j + 1)],
                                        snat_m[:, :, cc], ident[:])
                # pt4 f = (c4, ri, k) -> lhs f view (ri, c4, k)
                src = pt4[:].rearrange("p (c r k) -> p r c k",
                                       c=4, r=2, k=KC)
                eng = (nc.vector.tensor_copy if g4 % 2 == 0
                       else nc.scalar.copy)
                eng(lhs_v[:, :, 4 * g4:4 * (g4 + 1), :], src)

            for g4 in range(4):
                thunks.append(lambda g4=g4: transp_group(g4))

            pending = prev + [(lhs, fsb)]

            if bi == 0:
                # first processed block: keep lhs, DFT runs paired with
                # the next block
                return thunks, pending

            def dft_strip(s8):
                tail = bi == NBLK - 1
                bb = s8 * NLON
                if tail and s8 % 2 == 1:
                    # odd tail strips take E/O banks from the (now idle)
                    # stage-1 ps1 ring: alternating PSUM pools doubles
                    # the WAR distance per pool, so strips overlap deeper
                    ypE = ps1.tile([128, NE], dt.float32, tag="pb")
                    ypO = ps1.tile([128, NE], dt.float32, tag="pb")
                    yp = None
                    tE, oE, tO, oO = ypE, 0, ypO, 0
                else:
                    yp = ps2.tile([128, NLON], dt.float32, tag="yp")
                    tE, oE, tO, oO = yp, 0, yp, NE
                for j, (lh, fb) in enumerate(pending):
                    emit_dft(tE, oE, tO, oO, lh, fb, s8, first=(j == 0),
                             last=(not tail and j == len(pending) - 1))
                if tail:
                    # append acc (all earlier blocks) into the PSUM
                    # accumulation: yp += I^T @ acc
                    nc.tensor.matmul(tE[:, oE:oE + NE], ident[:],
                                     acc[:, bb:bb + NE],
                                     start=False, stop=True)
                    nc.tensor.matmul(tO[:, oO:oO + NE], ident[:],
                                     acc[:, bb + NE:bb + NLON],
                                     start=False, stop=True)
                a_sl = acc[:, bb:bb + NLON]
                if bi == 1:
                    eng = nc.vector.tensor_copy if s8 % 2 else nc.scalar.copy
                    eng(a_sl, yp[:])
                elif bi == 2:
                    nc.vector.tensor_add(a_sl, a_sl, yp[:])
                else:
                    # tail: the PE folds acc into the same PSUM group via
                    # identity matmuls, so E|O = full spectrum; the
                    # vector engines only evacuate + E/O-fold:
                    #   y[n] = E[n]+O[n], y[1024-n] = E[n]-O[n]
                    uv = uvp.tile([128, NLON], dt.float16, tag="uv")
                    nc.scalar.copy(uv[:, 0:NE], tE[:, oE:oE + NE])
                    nc.vector.tensor_copy(uv[:, NE:NLON],
                                          tO[:, oO:oO + NE])
                    ysb = ysp.tile([128, NLON], dt.float16, tag="ysb")
                    nc.gpsimd.tensor_add(
                        ysb[:, 1:NE], uv[:, 1:NE], uv[:, NE + 1:NLON])
                    nc.vector.tensor_sub(
                        ysb[:, NE + 1:NLON],
                        uv[:, NE - 1:0:-1], uv[:, NLON - 1:NE:-1])
                    nc.scalar.copy(ysb[:, 0:1], uv[:, 0:1])
                    nc.scalar.copy(ysb[:, NE:NE + 1], uv[:, NE:NE + 1])
                    # issue from the (idle) sync queue: an issue on scalar
                    # would make the next strip's ACT copy wait for this
                    # strip's full fold (in-order queue)
                    nc.sync.dma_start(
                        out=y[s8 * 128:(s8 + 1) * 128, :], in_=ysb[:])

            for s8 in range(8):
                thunks.append(lambda s8=s8: dft_strip(s8))
            return thunks, []

        pending = []
        for bi, b in enumerate(BORDER):
            # DFT matrix slice for this block (prefetched during stage 1)
            fsb = fp.tile([128, FW], dt.float16, tag="fsb")
            nc.sync.dma_start(out=fsb[:], in_=fmat[:, b * FW:(b + 1) * FW])

            # S^T staging for this 128-m block:
            #   partition = ri*64+k, free = m_loc*16 + c   (fp16)
            snat = snp.tile([128, 128 * C], dt.float16, tag="snat")
            snat_g = snat[:].rearrange("p (g s two c) -> p g s two c",
                                       g=8, s=8, two=2, c=C)
            snat_m = snat[:].rearrange("p (m c) -> p m c", c=C)

            # ---- stage 1: Legendre matmuls, 8 m-pairs per PSUM bank ----
            for g in range(8):
                G = b * 8 + g
                pos = BANK_SEQ.index(G)
                # issue every slab up to the lookahead horizon
                si_hi = min(_BANK_SLAB[min(pos + PREF_BANKS, NBANK - 1)],
                            NSLABS - 1)
                for si in range(si_hi + 1):
                    get_slab(si)
                # drain deferred units from the previous block FIRST:
                # their inputs are long ready, so the in-order PE fills
                # the wait for this bank's slab with useful work.  Drain
                # nothing at the last two banks: their extractions must
                # not queue behind deferred fold work on DVE/ACT, or the
                # next block's transposes stall on them (measured 2.2us
                # per block transition)
                for _ in range(DRAIN_N if g < DRAIN_CUT else 0):
                    if deferred:
                        deferred.pop(0)()
                pb = ps1.tile([128, 512], dt.float32, tag="pb")
                pb_v = pb[:].rearrange("p (s mj r c) -> p s mj r c",
                                       s=8, mj=2, r=2, c=C)
                ops = _BANK_OPS[G]
                for j, (t, l0, K, col) in enumerate(ops):
                    si_op = _slab_of(col)
                    st = get_slab(si_op)
                    o = (col - _SLAB_COL0[si_op]) * TILE_W
                    s = t % 8
                    if PCT_FP8:
                        lhsT = st[0:K, o:o + PCT_W16].bitcast(dt.float8e3)
                    else:
                        lhsT = st[0:K, o:o + PCT_W16]
                    mov = st[0:K, o + PCT_W16:o + TILE_W]
                    if X_FP8:
                        mov = mov.bitcast(dt.float8e3)
                    nc.tensor.matmul(
                        pb[:, s * 64:(s + 1) * 64],
                        lhsT,
                        mov,
                        start=(j == 0), stop=(j == len(ops) - 1),
                        tile_position=(0, 0))
                # extract diagonal (mi==mj) blocks -> snat (cast fp16),
                # split across DVE and ACT
                for mi in range(2):
                    for r in range(2):
                        eng = (nc.vector.tensor_copy if (mi + r) % 2 == 0
                               else nc.scalar.copy)
                        eng(snat_g[r * 64:(r + 1) * 64, g, :, mi, :],
                            pb_v[mi * 64:(mi + 1) * 64, :, mi, r, :])

            while deferred:
                deferred.pop(0)()
            deferred, pending = make_deferred(bi, snat_m, fsb, pending)

        # last block's work has no next block to hide in
        while deferred:
            deferred.pop(0)()

    nc.compile()
    return nc


def _build_fmat():
    m = np.arange(M_E)
    n2 = np.arange(NE)
    w = np.where(m == 0, 1.0, 2.0)
    ang = 2.0 * np.pi * np.outer(m, n2) / NLON
    wc = (w[:, None] * np.cos(ang)).astype(np.float16)     # E weights
    ws = (-w[:, None] * np.sin(ang)).astype(np.float16)    # O weights
    fz = (w * np.where(m % 2 == 0, 1.0, -1.0)).astype(np.float16)  # y[512]
    fmat = np.zeros((128, NBLK * FW + 128), np.float16)
    for b in range(NBLK):
        sl = slice(b * 128, (b + 1) * 128)
        fmat[:, b * FW:b * FW + NE] = wc[sl]
        fmat[:, b * FW + NE:b * FW + 2 * NE] = ws[sl]
        fmat[:, b * FW + 2 * NE] = fz[sl]
    fmat[:, NBLK * FW:] = np.eye(128, dtype=np.float16)
    return fmat


_ALL_OPS = [op for ops in _BANK_OPS for op in ops]


def _pack_streams(x_re, x_im, pct):
    """Per-core packed stream of shelf-packed (<=128 x TILE_W) tiles.
    Tile layout per row: [pct: PCT_W16 fp16 slots | x: 64 fp16]."""
    import ml_dtypes

    x_re = np.asarray(x_re, np.float32)
    x_im = np.asarray(x_im, np.float32)
    pct = np.asarray(pct, np.float32)

    # x part is core-independent: build once
    template = np.zeros((128, F_TOT), np.float16)
    tv = template.reshape(128, NCOLS, TILE_W)
    for (t, l0, K, col) in _ALL_OPS:
        xr = x_re[0, :, l0:l0 + K, 2 * t:2 * t + 2]   # (c, K, 2m)
        xi = x_im[0, :, l0:l0 + K, 2 * t:2 * t + 2]
        xx = np.stack([xr, xi], axis=0)                # (r, c, K, m)
        xk = xx.transpose(2, 3, 0, 1).reshape(K, 64)
        if X_FP8:
            xk = np.ascontiguousarray(
                xk.astype(ml_dtypes.float8_e3m4)).view(np.float16)
        tv[0:K, col, PCT_W16:] = xk

    streams = []
    for core in range(NCORES):
        k0 = core * KC
        sbuf = template.copy()
        sv = sbuf.reshape(128, NCOLS, TILE_W)
        for (t, l0, K, col) in _ALL_OPS:
            blk = pct[2 * t:2 * t + 2, k0:k0 + KC, l0:l0 + K]  # (2m, 64k, K)
            pk = blk.transpose(2, 0, 1).reshape(K, 128)
            if PCT_FP8:
                pk = np.ascontiguousarray(
                    pk.astype(ml_dtypes.float8_e3m4)).view(np.uint8)
                pk = pk.view(np.float16)               # (K, 64)
            sv[0:K, col, 0:PCT_W16] = pk
        streams.append(sbuf)
    return streams


_NC_CACHE = [None]


def _get_program():
    if _NC_CACHE[0] is None:
        _NC_CACHE[0] = build_program()
    return _NC_CACHE[0]


def run(x_re, x_im, pct, nlon=NLON, trace=False, trace_kwargs=None):
    from concourse.bass_utils import run_bass_kernel_spmd

    assert int(nlon) == NLON
    nc = _get_program()
    fmat = _build_fmat()
    streams = _pack_streams(x_re, x_im, pct)
    in_maps = [{"stream": streams[i], "fmat": fmat} for i in range(NCORES)]
    res = run_bass_kernel_spmd(nc, in_maps, list(range(NCORES)),
                               trace=trace, **(trace_kwargs or {}))
    out = np.empty((B, C, NLAT, NLON), np.float32)
    for core in range(NCORES):
        yc = res.results[core]["y"].astype(np.float32).reshape(C, KC, NLON)
        out[0, :, core * KC:(core + 1) * KC, :] = yc
    return out, res


def kernel(x_re, x_im, pct, nlon=NLON, **_unused):
    out, _ = run(x_re, x_im, pct, nlon)
    return out



# revision 93
# speedup vs baseline: 1.0052x; 1.0052x over previous
"""Distributed inverse real SHT on 8 Trainium2 NeuronCores (Bass/Tile).

Math (per reference):
    S[c,k,m]  = sum_l x[c,m,l] * pct[m,k,l]          (Legendre synthesis)
    y[c,k,n]  = irfft_{n=1024}(S, norm='forward')
              = sum_m  Sre[c,k,m]*Fc[m,n] + Sim[c,k,m]*Fs[m,n]
    with Fc[m,n] = w_m cos(2*pi*m*n/N), Fs[m,n] = -w_m sin(2*pi*m*n/N),
    w_0 = 1, w_m = 2 otherwise (verified exactly vs np.fft.irfft).
    pct[m,*,l] = 0 for l < m (triangular), and the m=512 row of pct is
    entirely zero (l < 512 always), so the effective mmax is 512.

Sharding: nlat (k) split across the 8 cores -> 64 output latitudes per
core, no inter-core communication.

Ideas scoped and rejected WITHOUT implementation (mechanism-level):
  - cross-core x broadcast via collectives: useless — the bound is
    per-core DMA-engine ingest (~235 GB/s, below the chip's per-core
    HBM fair share), and CC receives ride the same DMA engines, so
    per-core ingest bytes are unchanged.  Only sub-fp8 compression
    would cut bytes, and both 4-bit accuracy and DVE expand cost are
    fatal.
  - on-device fmat generation: saves ~1MB of sync-queue DMA (~2us)
    but needs PE outer-product + range reduction (mn mod 1024) + ACT
    sin/cos tables of unknown range support.
  - fmat (or its Im half) in fp8e3: 2.32e-2 (2.12e-2) total error,
    over the 2e-2 gate.
  - wc n-symmetry folding (ship half the E table): needs (-1)^m
    pre-applied to alternate lh partitions, +8us DVE for -0.7us DMA.

v6 addenda:
  - K_A2 (mirror split of the A-range at row 32 for K<=32 columns)
    measured +0.4us vs a back-to-back control despite saving the same
    0.39MB as K_B3: its extra issues land on the gpsimd queue (which
    paces slab readiness) and its split slabs sit in the startup
    region.  Left available but off by default.

v6 vs v5:
  - third DMA row-range: columns with K in (64,96] skip rows 96:128
    (0.39MB less stream, only 4 slabs split so issue overhead stays
    negligible; -0.6us vs a contemporaneous control).
  - slab ring depth stays 6 (8 measured +7us vs control: SBUF pressure,
    no slab-wait payoff).

v5 vs v4:
  - each slab's rows-64:128 range is issued from the sync queue while
    rows-0:64 stay on gpsimd: two queues feed the DMA engines
    concurrently (~1.5us).  NOTE: whole-slab queue alternation and
    per-slab contiguous DRAM tensors both HURT; only this row-range
    split helps.
  - A/B runs are only comparable within a machine-state window: the
    device band drifted +11us mid-session (thermal/co-tenant).  Re-test
    any surprising regression against a fresh same-window baseline.

v4 vs v3:
  - trailing 4 slabs are 1-bank (finer arrival granularity where the
    stage-1 tail chases the last slabs; 8 trailing 1-bank slabs or
    1-bank leading slabs measured no better).
  - odd tail strips take their E/O PSUM banks from the stage-1 ps1
    ring (same tag, same tile size, ring is idle by then): alternating
    PSUM pools doubles the WAR distance per pool in the tail.
  Additional variants measured and REJECTED: per-slab contiguous DRAM
  tensors (+11us: the DMA engines parallelize better over the
  monolithic strided layout), drain repacing 4-early/0-late (+4us) and
  2-per-bank (+4us), uv/ysb rings of 4 (+4us with K_PREF=10).

v3 vs v2:
  - x is streamed as fp8e3 (e3m4) too, not just pct: the Legendre
    matmul runs fp8e3 x fp8e3.  Cuts stream DMA from ~17.6MB to
    ~13.2MB per core.  Max-rel error 1.92e-2 (gate 2e-2, deterministic
    inputs, verified offline and on hardware).
  - Slab plan generalized: banks-per-slab list + lookahead prefetch
    are tunable (K_SLABS / K_PREF); measured optimum stays at uniform
    2-bank slabs with ~8-bank lookahead.
  Scheduling variants measured and REJECTED (each +3..18us):
  equal-byte 1.5MB slabs (consumers wait whole-slab -> coarse
  pipeline), 4-bank slabs for thin blocks, alternating slab issue
  queues, 3-block PSUM DFT grouping, PE id-seed of block1's group,
  folds reading PSUM directly (tail is vector-bound; GPS cannot read
  PSUM, TT ops allow only one PSUM input), tail thunk interleaving.

v2 vs v1:
  - pct streamed as fp8e3 (e3m4) instead of fp16.  Verified offline
    against the reference: max-rel error 1.34e-2 (gate 2e-2).
  - Slab columns sorted K-descending and DMA'd as two row-ranges
    (rows [64:128) only for columns with K > 64), removing most of the
    20% DMA row padding of v1.  (Nonzero-tile_position shelf packing
    hangs this hardware, so row-skip DMA is the packing mechanism.)
  - DFT restructured: blocks (3,2) accumulate in one PSUM group (one
    SBUF copy, no add), block 1 adds, block 0 folds directly from
    PSUM+acc in the tail.  Halves the DVE accumulate traffic; the fp32
    "acc" buffer is now fp16.
  - Slab DMAs issued from the GpSimd queue and y writeback from the
    Scalar queue to unload the saturated Sync engine.
"""

import os
import numpy as np
from contextlib import ExitStack


NLAT, NLON = 512, 1024
LMAX, MMAX = 512, 513
M_E = 512            # effective mmax (m=512 row of pct is identically zero)
B, C = 1, 16
NCORES = 8
KC = NLAT // NCORES  # 64 latitudes per core
PAIRS = M_E // 2     # 256 m-pairs
NBLK = 4             # 128-m blocks
NBANK = PAIRS // 8   # 32 PSUM banks (8 pairs each)

PCT_FP8 = os.environ.get("K_FP8", "1") == "1"  # pct as fp8e3, 2/fp16-slot
X_FP8 = os.environ.get("K_XFP8", "1") == "1"   # x as fp8e3 too
ALT_DMAQ = os.environ.get("K_DMAQ", "1") == "1"  # slab/y DMA off sync eng
B_ON_SYNC = os.environ.get("K_BSYNC", "1") == "1"  # slab rows 64:128 on sync
SP_BUFS = int(os.environ.get("K_SPBUFS", "6"))     # slab ring depth
B_3RANGE = os.environ.get("K_B3", "1") == "1"      # skip rows 96:128 K<=96
A_2RANGE = os.environ.get("K_A2", "0") == "1"      # skip rows 32:64 K<=32
UV_BUFS = int(os.environ.get("K_UVB", "3"))        # tail uv/ysb ring depth
# (ring depth 8 measured +7us against a contemporaneous control:
# deeper prefetch adds SBUF pressure with no slab-wait payoff)
PCT_W16 = 64 if PCT_FP8 else 128   # fp16 slots for the pct part of a tile
X_W16 = 32 if X_FP8 else 64        # fp16 slots for the x part (2m*2ri*16c)
TILE_W = PCT_W16 + X_W16

# processing order: shortest pairs first; each block's deferred
# transpose+DFT work hides inside the next block's DMA window
BORDER = [3, 2, 1, 0]
BANK_SEQ = [b * 8 + g for b in BORDER for g in range(8)]


PREF_BANKS = int(os.environ.get("K_PREF", "8"))  # bank lookahead
DRAIN_N = int(os.environ.get("K_DRAIN", "3"))    # deferred drains per bank
DRAIN_CUT = int(os.environ.get("K_DRAINCUT", "8"))  # no drains from here
# banks per slab along the processing order (block3, block2, block1, block0)
SLAB_BANKS = [int(c) for c in os.environ.get(
    "K_SLABS", ",".join(["2"] * 14 + ["1"] * 4)).split(",")]
assert sum(SLAB_BANKS) == NBANK


def _plan():
    """One 128-row column per (pair, l-chunk) tile.  Columns are grouped
    into slabs of ~equal DMA bytes (not equal bank counts): thin-column
    blocks pack more columns per slab, which keeps the per-partition DMA
    packet size large (~w*192B) and the DMA cadence uniform.  Within
    each slab columns are sorted K-descending so the slab DMA can skip
    rows [64:128) for the trailing columns with K <= 64.

    Returns (bank_ops, slab_col0, slab_w64, bank_slab, ncols):
    bank_ops[G] = [(t, l0, K, col)] K-descending (first op start=True);
    slab_w64[si] = column count needing rows >= 64; bank_slab[p] = max
    slab index used by the bank at processing position p."""
    # columns in processing order, with bank position attached
    cols = []
    for pos, G in enumerate(BANK_SEQ):
        for t in range(8 * G, 8 * G + 8):
            l0 = 2 * t
            L = LMAX - l0
            nch = (L + 127) // 128
            for c in range(nch):
                cols.append((pos, G, t, l0 + 128 * c,
                             min(128, L - 128 * c)))
    # slab boundaries aligned to bank-processing positions: fine slabs
    # where consumption is fast, 4-bank slabs for the thin-column blocks
    # (3, 2) so the per-partition DMA packet stays large
    pos_bounds = []
    p = 0
    for nb in SLAB_BANKS:
        p += nb
        pos_bounds.append(p)
    bounds = []
    bi_ = 0
    for i, (pos, G, t, l0, K) in enumerate(cols):
        if pos >= pos_bounds[bi_]:
            bounds.append(i)
            bi_ += 1
    bounds.append(len(cols))
    bank_ops = [[] for _ in range(NBANK)]
    slab_col0 = [0]
    slab_w64 = []
    slab_w96 = []
    slab_w32 = []
    bank_slab = [0] * NBANK
    start = 0
    for si, end in enumerate(bounds):
        tiles = sorted(cols[start:end], key=lambda x: -x[4])
        slab_w64.append(sum(1 for x in tiles if x[4] > 64))
        slab_w96.append(sum(1 for x in tiles if x[4] > 96))
        slab_w32.append(sum(1 for x in tiles if x[4] > 32))
        for i, (pos, G, t, l0, K) in enumerate(tiles):
            bank_ops[G].append((t, l0, K, slab_col0[-1] + i))
            bank_slab[pos] = max(bank_slab[pos], si)
        slab_col0.append(slab_col0[-1] + len(tiles))
        start = end
    import bisect

    def slab_of(col):
        return bisect.bisect_right(slab_col0, col) - 1

    for G in range(NBANK):
        # earlier-slab ops first so available work runs before any stall
        bank_ops[G].sort(key=lambda x: (slab_of(x[3]), -x[2]))
    return (bank_ops, slab_col0, slab_w64, slab_w96, slab_w32,
            bank_slab, len(cols))


(_BANK_OPS, _SLAB_COL0, _SLAB_W64, _SLAB_W96, _SLAB_W32, _BANK_SLAB,
 NCOLS) = _plan()
NSLABS = len(_SLAB_W64)
F_TOT = NCOLS * TILE_W


def _slab_of(col):
    import bisect
    return bisect.bisect_right(_SLAB_COL0, col) - 1

# Even/odd DFT folding: E[n'] = sum_m wc*Re and O[n'] = sum_m ws*Im
# for n' in [0,512) plus the y[512] column (folded into O's zero column);
# then y[n'] = E+O, y[1024-n'] = E-O.
NE = NLON // 2       # 512
FW = NE + NE + 16    # wc | ws | (y512 col + pad)
CKC = C * KC         # 1024


def build_program():
    from concourse import bacc, bass, masks, mybir, tile

    dt = mybir.dt
    nc = bacc.Bacc("TRN2", target_bir_lowering=False, debug=False,
                   num_devices=NCORES)

    # NOTE: per-slab contiguous DRAM tensors were measured 11us SLOWER
    # than this monolithic strided layout (the engines parallelize
    # better over uniform big-stride partition rows)
    stream = nc.dram_tensor("stream", [128, F_TOT], dt.float16,
                            kind="ExternalInput")
    fmat = nc.dram_tensor("fmat", [128, NBLK * FW + 128], dt.float16,
                          kind="ExternalInput")
    y = nc.dram_tensor("y", [CKC, NLON], dt.float16, kind="ExternalOutput")

    with tile.TileContext(nc) as tc, ExitStack() as ctx:
        sp = ctx.enter_context(
            tc.tile_pool(name="stream", bufs=SP_BUFS))
        cp = ctx.enter_context(tc.tile_pool(name="const", bufs=1))
        fp = ctx.enter_context(tc.tile_pool(name="fsb", bufs=4))
        ysp = ctx.enter_context(tc.tile_pool(name="ysb", bufs=UV_BUFS))
        uvp = ctx.enter_context(tc.tile_pool(name="uv", bufs=UV_BUFS))
        snp = ctx.enter_context(tc.tile_pool(name="snat", bufs=2))
        lhp = ctx.enter_context(tc.tile_pool(name="lhs", bufs=2))
        ps1 = ctx.enter_context(
            tc.tile_pool(name="ps1", bufs=3, space=bass.MemorySpace.PSUM))
        pst = ctx.enter_context(
            tc.tile_pool(name="pst", bufs=1, space=bass.MemorySpace.PSUM))
        ps2 = ctx.enter_context(
            tc.tile_pool(name="ps2", bufs=2, space=bass.MemorySpace.PSUM))

        # fp16 partial accumulator for the blocks handled before the tail:
        # partition = (c2,k64) within strip, free = strip*1024 + n
        acc = cp.tile([128, 8 * NLON], dt.float16)

        slabs = {}

        def get_slab(si, eng=None):
            si = min(si, NSLABS - 1)
            if si not in slabs:
                w = _SLAB_COL0[si + 1] - _SLAB_COL0[si]
                w64 = _SLAB_W64[si]
                st = sp.tile([128, w * TILE_W], dt.float16, tag="slab")
                o0 = _SLAB_COL0[si] * TILE_W
                if eng is None:
                    eng = nc.gpsimd if ALT_DMAQ else nc.sync
                # rows 64:128 go through the sync queue so two queues
                # feed the DMA engines concurrently (K_BSYNC=0 reverts)
                eng_b = nc.sync if B_ON_SYNC else eng
                w32 = _SLAB_W32[si]
                if A_2RANGE and w32 < w:
                    # columns with K <= 32 skip rows 32:64; the kept
                    # 32:64 piece rides sync so the split slab's pieces
                    # land in parallel across queues
                    eng.dma_start(
                        out=st[0:32, :],
                        in_=stream[0:32, o0:o0 + w * TILE_W])
                    if w32:
                        eng_b.dma_start(
                            out=st[32:64, 0:w32 * TILE_W],
                            in_=stream[32:64, o0:o0 + w32 * TILE_W])
                else:
                    eng.dma_start(
                        out=st[0:64, :],
                        in_=stream[0:64, o0:o0 + w * TILE_W])
                w96 = _SLAB_W96[si]
                if B_3RANGE and w64 and w96 < w64:
                    # columns with K in (64,96] skip rows 96:128 too
                    eng_b.dma_start(
                        out=st[64:96, 0:w64 * TILE_W],
                        in_=stream[64:96, o0:o0 + w64 * TILE_W])
                    if w96:
                        eng_b.dma_start(
                            out=st[96:128, 0:w96 * TILE_W],
                            in_=stream[96:128, o0:o0 + w96 * TILE_W])
                elif w64:
                    eng_b.dma_start(
                        out=st[64:128, 0:w64 * TILE_W],
                        in_=stream[64:128, o0:o0 + w64 * TILE_W])
                slabs[si] = st
            return slabs[si]

        # first slabs from the sync queue (ready before gpsimd's table
        # load); identity comes in with fmat instead of being computed
        # (slab 1 on gpsimd instead measured +5us-or-drift; reverted)
        get_slab(0, nc.sync)
        get_slab(1, nc.sync)
        get_slab(2, nc.sync)

        ident = cp.tile([128, 128], dt.float16)
        nc.sync.dma_start(out=ident[:],
                          in_=fmat[:, NBLK * FW:NBLK * FW + 128])

        deferred = []  # previous block's transpose + DFT work, as thunks

        def emit_dft(tE, oE, tO, oO, lh, fsb, s8, first, last):
            """One block's 3 DFT matmuls for strip s8 into the E bank
            (tile tE cols oE:oE+NE) and O bank (tO cols oO:oO+NE, which
            also takes the y512 column).  Each bank is one accumulation
            group."""
            l0 = lh[:, s8 * 128:s8 * 128 + 128]               # Re rows
            l1 = lh[:, CKC + s8 * 128:CKC + s8 * 128 + 128]   # Im rows
            nc.tensor.matmul(tE[:, oE:oE + NE], l0, fsb[:, 0:NE],
                             start=first, stop=last)
            nc.tensor.matmul(tO[:, oO:oO + NE], l1, fsb[:, NE:2 * NE],
                             start=first, stop=False)
            nc.tensor.matmul(tO[:, oO:oO + 8], l0,
                             fsb[:, 2 * NE:2 * NE + 8],
                             start=False, stop=last)

        def make_deferred(bi, snat_m, fsb, prev):
            """Block's post-stage-1 work: PE transposes into the stage-2
            lhsT layout, then (depending on position in the schedule)
            the DFT matmuls + accumulation.  prev = list of (lhs, fsb)
            for blocks whose DFT is still pending."""
            thunks = []
            lhs = lhp.tile([128, 2 * CKC], dt.float16, tag="lhs")
            lhs_v = lhs[:].rearrange("p (r c k) -> p r c k", r=2, c=C, k=KC)

            def transp_group(g4):
                # 4 channels' PE transposes into one PSUM tile, then a
                # single strided copy into the lhs layout
                pt4 = pst.tile([128, 512], dt.float16, tag="pt")
                for j in range(4):
                    cc = 4 * g4 + j
                    nc.tensor.transpose(pt4[:, 128 * j:128 * (j + 1)],
                                        snat_m[:, :, cc], ident[:])
                # pt4 f = (c4, ri, k) -> lhs f view (ri, c4, k)
                src = pt4[:].rearrange("p (c r k) -> p r c k",
                                       c=4, r=2, k=KC)
                eng = (nc.vector.tensor_copy if g4 % 2 == 0
                       else nc.scalar.copy)
                eng(lhs_v[:, :, 4 * g4:4 * (g4 + 1), :], src)

            for g4 in range(4):
                thunks.append(lambda g4=g4: transp_group(g4))

            pending = prev + [(lhs, fsb)]

            if bi == 0:
                # first processed block: keep lhs, DFT runs paired with
                # the next block
                return thunks, pending

            def dft_strip(s8):
                tail = bi == NBLK - 1
                bb = s8 * NLON
                if tail and s8 % 2 == 1:
                    # odd tail strips take E/O banks from the (now idle)
                    # stage-1 ps1 ring: alternating PSUM pools doubles
                    # the WAR distance per pool, so strips overlap deeper
                    ypE = ps1.tile([128, NE], dt.float32, tag="pb")
                    ypO = ps1.tile([128, NE], dt.float32, tag="pb")
                    yp = None
                    tE, oE, tO, oO = ypE, 0, ypO, 0
                else:
                    yp = ps2.tile([128, NLON], dt.float32, tag="yp")
                    tE, oE, tO, oO = yp, 0, yp, NE
                for j, (lh, fb) in enumerate(pending):
                    emit_dft(tE, oE, tO, oO, lh, fb, s8, first=(j == 0),
                             last=(not tail and j == len(pending) - 1))
                if tail:
                    # append acc (all earlier blocks) into the PSUM
                    # accumulation: yp += I^T @ acc
                    nc.tensor.matmul(tE[:, oE:oE + NE], ident[:],
                                     acc[:, bb:bb + NE],
                                     start=False, stop=True)
                    nc.tensor.matmul(tO[:, oO:oO + NE], ident[:],
                                     acc[:, bb + NE:bb + NLON],
                                     start=False, stop=True)
                a_sl = acc[:, bb:bb + NLON]
                if bi == 1:
                    eng = nc.vector.tensor_copy if s8 % 2 else nc.scalar.copy
                    eng(a_sl, yp[:])
                elif bi == 2:
                    nc.vector.tensor_add(a_sl, a_sl, yp[:])
                else:
                    # tail: the PE folds acc into the same PSUM group via
                    # identity matmuls, so E|O = full spectrum; the
                    # vector engines only evacuate + E/O-fold:
                    #   y[n] = E[n]+O[n], y[1024-n] = E[n]-O[n]
                    uv = uvp.tile([128, NLON], dt.float16, tag="uv")
                    nc.scalar.copy(uv[:, 0:NE], tE[:, oE:oE + NE])
                    nc.vector.tensor_copy(uv[:, NE:NLON],
                                          tO[:, oO:oO + NE])
                    ysb = ysp.tile([128, NLON], dt.float16, tag="ysb")
                    nc.gpsimd.tensor_add(
                        ysb[:, 1:NE], uv[:, 1:NE], uv[:, NE + 1:NLON])
                    nc.vector.tensor_sub(
                        ysb[:, NE + 1:NLON],
                        uv[:, NE - 1:0:-1], uv[:, NLON - 1:NE:-1])
                    nc.scalar.copy(ysb[:, 0:1], uv[:, 0:1])
                    nc.scalar.copy(ysb[:, NE:NE + 1], uv[:, NE:NE + 1])
                    # issue from the (idle) sync queue: an issue on scalar
                    # would make the next strip's ACT copy wait for this
                    # strip's full fold (in-order queue)
                    nc.sync.dma_start(
                        out=y[s8 * 128:(s8 + 1) * 128, :], in_=ysb[:])

            for s8 in range(8):
                thunks.append(lambda s8=s8: dft_strip(s8))
            return thunks, []

        pending = []
        for bi, b in enumerate(BORDER):
            # DFT matrix slice for this block (prefetched during stage 1)
            fsb = fp.tile([128, FW], dt.float16, tag="fsb")
            nc.sync.dma_start(out=fsb[:], in_=fmat[:, b * FW:(b + 1) * FW])

            # S^T staging for this 128-m block:
            #   partition = ri*64+k, free = m_loc*16 + c   (fp16)
            snat = snp.tile([128, 128 * C], dt.float16, tag="snat")
            snat_g = snat[:].rearrange("p (g s two c) -> p g s two c",
                                       g=8, s=8, two=2, c=C)
            snat_m = snat[:].rearrange("p (m c) -> p m c", c=C)

            # ---- stage 1: Legendre matmuls, 8 m-pairs per PSUM bank ----
            for g in range(8):
                G = b * 8 + g
                pos = BANK_SEQ.index(G)
                # issue every slab up to the lookahead horizon
                si_hi = min(_BANK_SLAB[min(pos + PREF_BANKS, NBANK - 1)],
                            NSLABS - 1)
                for si in range(si_hi + 1):
                    get_slab(si)
                # drain deferred units from the previous block FIRST:
                # their inputs are long ready, so the in-order PE fills
                # the wait for this bank's slab with useful work.  Drain
                # nothing at the last two banks: their extractions must
                # not queue behind deferred fold work on DVE/ACT, or the
                # next block's transposes stall on them (measured 2.2us
                # per block transition)
                for _ in range(DRAIN_N if g < DRAIN_CUT else 0):
                    if deferred:
                        deferred.pop(0)()
                pb = ps1.tile([128, 512], dt.float32, tag="pb")
                pb_v = pb[:].rearrange("p (s mj r c) -> p s mj r c",
                                       s=8, mj=2, r=2, c=C)
                ops = _BANK_OPS[G]
                for j, (t, l0, K, col) in enumerate(ops):
                    si_op = _slab_of(col)
                    st = get_slab(si_op)
                    o = (col - _SLAB_COL0[si_op]) * TILE_W
                    s = t % 8
                    if PCT_FP8:
                        lhsT = st[0:K, o:o + PCT_W16].bitcast(dt.float8e3)
                    else:
                        lhsT = st[0:K, o:o + PCT_W16]
                    mov = st[0:K, o + PCT_W16:o + TILE_W]
                    if X_FP8:
                        mov = mov.bitcast(dt.float8e3)
                    nc.tensor.matmul(
                        pb[:, s * 64:(s + 1) * 64],
                        lhsT,
                        mov,
                        start=(j == 0), stop=(j == len(ops) - 1),
                        tile_position=(0, 0))
                # extract diagonal (mi==mj) blocks -> snat (cast fp16),
                # split across DVE and ACT
                for mi in range(2):
                    for r in range(2):
                        eng = (nc.vector.tensor_copy if (mi + r) % 2 == 0
                               else nc.scalar.copy)
                        eng(snat_g[r * 64:(r + 1) * 64, g, :, mi, :],
                            pb_v[mi * 64:(mi + 1) * 64, :, mi, r, :])

            while deferred:
                deferred.pop(0)()
            deferred, pending = make_deferred(bi, snat_m, fsb, pending)

        # last block's work has no next block to hide in
        while deferred:
            deferred.pop(0)()

    nc.compile()
    return nc


def _build_fmat():
    m = np.arange(M_E)
    n2 = np.arange(NE)
    w = np.where(m == 0, 1.0, 2.0)
    ang = 2.0 * np.pi * np.outer(m, n2) / NLON
    wc = (w[:, None] * np.cos(ang)).astype(np.float16)     # E weights
    ws = (-w[:, None] * np.sin(ang)).astype(np.float16)    # O weights
    fz = (w * np.where(m % 2 == 0, 1.0, -1.0)).astype(np.float16)  # y[512]
    fmat = np.zeros((128, NBLK * FW + 128), np.float16)
    for b in range(NBLK):
        sl = slice(b * 128, (b + 1) * 128)
        fmat[:, b * FW:b * FW + NE] = wc[sl]
        fmat[:, b * FW + NE:b * FW + 2 * NE] = ws[sl]
        fmat[:, b * FW + 2 * NE] = fz[sl]
    fmat[:, NBLK * FW:] = np.eye(128, dtype=np.float16)
    return fmat


_ALL_OPS = [op for ops in _BANK_OPS for op in ops]


def _pack_streams(x_re, x_im, pct):
    """Per-core packed stream of shelf-packed (<=128 x TILE_W) tiles.
    Tile layout per row: [pct: PCT_W16 fp16 slots | x: 64 fp16]."""
    import ml_dtypes

    x_re = np.asarray(x_re, np.float32)
    x_im = np.asarray(x_im, np.float32)
    pct = np.asarray(pct, np.float32)

    # x part is core-independent: build once
    template = np.zeros((128, F_TOT), np.float16)
    tv = template.reshape(128, NCOLS, TILE_W)
    for (t, l0, K, col) in _ALL_OPS:
        xr = x_re[0, :, l0:l0 + K, 2 * t:2 * t + 2]   # (c, K, 2m)
        xi = x_im[0, :, l0:l0 + K, 2 * t:2 * t + 2]
        xx = np.stack([xr, xi], axis=0)                # (r, c, K, m)
        xk = xx.transpose(2, 3, 0, 1).reshape(K, 64)
        if X_FP8:
            xk = np.ascontiguousarray(
                xk.astype(ml_dtypes.float8_e3m4)).view(np.float16)
        tv[0:K, col, PCT_W16:] = xk

    streams = []
    for core in range(NCORES):
        k0 = core * KC
        sbuf = template.copy()
        sv = sbuf.reshape(128, NCOLS, TILE_W)
        for (t, l0, K, col) in _ALL_OPS:
            blk = pct[2 * t:2 * t + 2, k0:k0 + KC, l0:l0 + K]  # (2m, 64k, K)
            pk = blk.transpose(2, 0, 1).reshape(K, 128)
            if PCT_FP8:
                pk = np.ascontiguousarray(
                    pk.astype(ml_dtypes.float8_e3m4)).view(np.uint8)
                pk = pk.view(np.float16)               # (K, 64)
            sv[0:K, col, 0:PCT_W16] = pk
        streams.append(sbuf)
    return streams


_NC_CACHE = [None]


def _get_program():
    if _NC_CACHE[0] is None:
        _NC_CACHE[0] = build_program()
    return _NC_CACHE[0]


def run(x_re, x_im, pct, nlon=NLON, trace=False, trace_kwargs=None):
    from concourse.bass_utils import run_bass_kernel_spmd

    assert int(nlon) == NLON
    nc = _get_program()
    fmat = _build_fmat()
    streams = _pack_streams(x_re, x_im, pct)
    in_maps = [{"stream": streams[i], "fmat": fmat} for i in range(NCORES)]
    res = run_bass_kernel_spmd(nc, in_maps, list(range(NCORES)),
                               trace=trace, **(trace_kwargs or {}))
    out = np.empty((B, C, NLAT, NLON), np.float32)
    for core in range(NCORES):
        yc = res.results[core]["y"].astype(np.float32).reshape(C, KC, NLON)
        out[0, :, core * KC:(core + 1) * KC, :] = yc
    return out, res


def kernel(x_re, x_im, pct, nlon=NLON, **_unused):
    out, _ = run(x_re, x_im, pct, nlon)
    return out



# revision 95
# speedup vs baseline: 1.0180x; 1.0128x over previous
"""Distributed inverse real SHT on 8 Trainium2 NeuronCores (Bass/Tile).

Math (per reference):
    S[c,k,m]  = sum_l x[c,m,l] * pct[m,k,l]          (Legendre synthesis)
    y[c,k,n]  = irfft_{n=1024}(S, norm='forward')
              = sum_m  Sre[c,k,m]*Fc[m,n] + Sim[c,k,m]*Fs[m,n]
    with Fc[m,n] = w_m cos(2*pi*m*n/N), Fs[m,n] = -w_m sin(2*pi*m*n/N),
    w_0 = 1, w_m = 2 otherwise (verified exactly vs np.fft.irfft).
    pct[m,*,l] = 0 for l < m (triangular), and the m=512 row of pct is
    entirely zero (l < 512 always), so the effective mmax is 512.

Sharding: nlat (k) split across the 8 cores -> 64 output latitudes per
core, no inter-core communication.

Ideas scoped and rejected WITHOUT implementation (mechanism-level):
  - cross-core x broadcast via collectives: useless — the bound is
    per-core DMA-engine ingest (~235 GB/s, below the chip's per-core
    HBM fair share), and CC receives ride the same DMA engines, so
    per-core ingest bytes are unchanged.  Only sub-fp8 compression
    would cut bytes, and both 4-bit accuracy and DVE expand cost are
    fatal.
  - on-device fmat generation: saves ~1MB of sync-queue DMA (~2us)
    but needs PE outer-product + range reduction (mn mod 1024) + ACT
    sin/cos tables of unknown range support.
  - fmat (or its Im half) in fp8e3: 2.32e-2 (2.12e-2) total error,
    over the 2e-2 gate.
  - wc n-symmetry folding (ship half the E table): needs (-1)^m
    pre-applied to alternate lh partitions, +8us DVE for -0.7us DMA.

v6 addenda:
  - K_A2 (mirror split of the A-range at row 32 for K<=32 columns)
    measured +0.4us vs a back-to-back control despite saving the same
    0.39MB as K_B3: its extra issues land on the gpsimd queue (which
    paces slab readiness) and its split slabs sit in the startup
    region.  Left available but off by default.

v6 vs v5:
  - third DMA row-range: columns with K in (64,96] skip rows 96:128
    (0.39MB less stream, only 4 slabs split so issue overhead stays
    negligible; -0.6us vs a contemporaneous control).
  - slab ring depth stays 6 (8 measured +7us vs control: SBUF pressure,
    no slab-wait payoff).

v5 vs v4:
  - each slab's rows-64:128 range is issued from the sync queue while
    rows-0:64 stay on gpsimd: two queues feed the DMA engines
    concurrently (~1.5us).  NOTE: whole-slab queue alternation and
    per-slab contiguous DRAM tensors both HURT; only this row-range
    split helps.
  - A/B runs are only comparable within a machine-state window: the
    device band drifted +11us mid-session (thermal/co-tenant).  Re-test
    any surprising regression against a fresh same-window baseline.

v4 vs v3:
  - trailing 4 slabs are 1-bank (finer arrival granularity where the
    stage-1 tail chases the last slabs; 8 trailing 1-bank slabs or
    1-bank leading slabs measured no better).
  - odd tail strips take their E/O PSUM banks from the stage-1 ps1
    ring (same tag, same tile size, ring is idle by then): alternating
    PSUM pools doubles the WAR distance per pool in the tail.
  Additional variants measured and REJECTED: per-slab contiguous DRAM
  tensors (+11us: the DMA engines parallelize better over the
  monolithic strided layout), drain repacing 4-early/0-late (+4us) and
  2-per-bank (+4us), uv/ysb rings of 4 (+4us with K_PREF=10).

v3 vs v2:
  - x is streamed as fp8e3 (e3m4) too, not just pct: the Legendre
    matmul runs fp8e3 x fp8e3.  Cuts stream DMA from ~17.6MB to
    ~13.2MB per core.  Max-rel error 1.92e-2 (gate 2e-2, deterministic
    inputs, verified offline and on hardware).
  - Slab plan generalized: banks-per-slab list + lookahead prefetch
    are tunable (K_SLABS / K_PREF); measured optimum stays at uniform
    2-bank slabs with ~8-bank lookahead.
  Scheduling variants measured and REJECTED (each +3..18us):
  equal-byte 1.5MB slabs (consumers wait whole-slab -> coarse
  pipeline), 4-bank slabs for thin blocks, alternating slab issue
  queues, 3-block PSUM DFT grouping, PE id-seed of block1's group,
  folds reading PSUM directly (tail is vector-bound; GPS cannot read
  PSUM, TT ops allow only one PSUM input), tail thunk interleaving.

v2 vs v1:
  - pct streamed as fp8e3 (e3m4) instead of fp16.  Verified offline
    against the reference: max-rel error 1.34e-2 (gate 2e-2).
  - Slab columns sorted K-descending and DMA'd as two row-ranges
    (rows [64:128) only for columns with K > 64), removing most of the
    20% DMA row padding of v1.  (Nonzero-tile_position shelf packing
    hangs this hardware, so row-skip DMA is the packing mechanism.)
  - DFT restructured: blocks (3,2) accumulate in one PSUM group (one
    SBUF copy, no add), block 1 adds, block 0 folds directly from
    PSUM+acc in the tail.  Halves the DVE accumulate traffic; the fp32
    "acc" buffer is now fp16.
  - Slab DMAs issued from the GpSimd queue and y writeback from the
    Scalar queue to unload the saturated Sync engine.
"""

import os
import numpy as np
from contextlib import ExitStack


NLAT, NLON = 512, 1024
LMAX, MMAX = 512, 513
M_E = 512            # effective mmax (m=512 row of pct is identically zero)
B, C = 1, 16
NCORES = 8
KC = NLAT // NCORES  # 64 latitudes per core
PAIRS = M_E // 2     # 256 m-pairs
NBLK = 4             # 128-m blocks
NBANK = PAIRS // 8   # 32 PSUM banks (8 pairs each)

PCT_FP8 = os.environ.get("K_FP8", "1") == "1"  # pct as fp8e3, 2/fp16-slot
X_FP8 = os.environ.get("K_XFP8", "1") == "1"   # x as fp8e3 too
ALT_DMAQ = os.environ.get("K_DMAQ", "1") == "1"  # slab/y DMA off sync eng
B_ON_SYNC = os.environ.get("K_BSYNC", "1") == "1"  # slab rows 64:128 on sync
SP_BUFS = int(os.environ.get("K_SPBUFS", "6"))     # slab ring depth
B_3RANGE = os.environ.get("K_B3", "1") == "1"      # skip rows 96:128 K<=96
A_2RANGE = os.environ.get("K_A2", "0") == "1"      # skip rows 32:64 K<=32
UV_BUFS = int(os.environ.get("K_UVB", "3"))        # tail uv/ysb ring depth
B_TAIL_GPS = int(os.environ.get("K_BTAIL", "0"))   # trailing B's on gpsimd
# (ring depth 8 measured +7us against a contemporaneous control:
# deeper prefetch adds SBUF pressure with no slab-wait payoff)
PCT_W16 = 64 if PCT_FP8 else 128   # fp16 slots for the pct part of a tile
X_W16 = 32 if X_FP8 else 64        # fp16 slots for the x part (2m*2ri*16c)
TILE_W = PCT_W16 + X_W16

# processing order: shortest pairs first; each block's deferred
# transpose+DFT work hides inside the next block's DMA window
BORDER = [3, 2, 1, 0]
BANK_SEQ = [b * 8 + g for b in BORDER for g in range(8)]


PREF_BANKS = int(os.environ.get("K_PREF", "8"))  # bank lookahead
DRAIN_N = int(os.environ.get("K_DRAIN", "3"))    # deferred drains per bank
DRAIN_CUT = int(os.environ.get("K_DRAINCUT", "8"))  # no drains from here
# banks per slab along the processing order (block3, block2, block1, block0)
SLAB_BANKS = [int(c) for c in os.environ.get(
    "K_SLABS", ",".join(["2"] * 14 + ["1"] * 4)).split(",")]
assert sum(SLAB_BANKS) == NBANK


def _plan():
    """One 128-row column per (pair, l-chunk) tile.  Columns are grouped
    into slabs of ~equal DMA bytes (not equal bank counts): thin-column
    blocks pack more columns per slab, which keeps the per-partition DMA
    packet size large (~w*192B) and the DMA cadence uniform.  Within
    each slab columns are sorted K-descending so the slab DMA can skip
    rows [64:128) for the trailing columns with K <= 64.

    Returns (bank_ops, slab_col0, slab_w64, bank_slab, ncols):
    bank_ops[G] = [(t, l0, K, col)] K-descending (first op start=True);
    slab_w64[si] = column count needing rows >= 64; bank_slab[p] = max
    slab index used by the bank at processing position p."""
    # columns in processing order, with bank position attached
    cols = []
    for pos, G in enumerate(BANK_SEQ):
        for t in range(8 * G, 8 * G + 8):
            l0 = 2 * t
            L = LMAX - l0
            nch = (L + 127) // 128
            for c in range(nch):
                cols.append((pos, G, t, l0 + 128 * c,
                             min(128, L - 128 * c)))
    # slab boundaries aligned to bank-processing positions: fine slabs
    # where consumption is fast, 4-bank slabs for the thin-column blocks
    # (3, 2) so the per-partition DMA packet stays large
    pos_bounds = []
    p = 0
    for nb in SLAB_BANKS:
        p += nb
        pos_bounds.append(p)
    bounds = []
    bi_ = 0
    for i, (pos, G, t, l0, K) in enumerate(cols):
        if pos >= pos_bounds[bi_]:
            bounds.append(i)
            bi_ += 1
    bounds.append(len(cols))
    bank_ops = [[] for _ in range(NBANK)]
    slab_col0 = [0]
    slab_w64 = []
    slab_w96 = []
    slab_w32 = []
    bank_slab = [0] * NBANK
    start = 0
    for si, end in enumerate(bounds):
        tiles = sorted(cols[start:end], key=lambda x: -x[4])
        slab_w64.append(sum(1 for x in tiles if x[4] > 64))
        slab_w96.append(sum(1 for x in tiles if x[4] > 96))
        slab_w32.append(sum(1 for x in tiles if x[4] > 32))
        for i, (pos, G, t, l0, K) in enumerate(tiles):
            bank_ops[G].append((t, l0, K, slab_col0[-1] + i))
            bank_slab[pos] = max(bank_slab[pos], si)
        slab_col0.append(slab_col0[-1] + len(tiles))
        start = end
    import bisect

    def slab_of(col):
        return bisect.bisect_right(slab_col0, col) - 1

    for G in range(NBANK):
        # earlier-slab ops first so available work runs before any stall
        bank_ops[G].sort(key=lambda x: (slab_of(x[3]), -x[2]))
    return (bank_ops, slab_col0, slab_w64, slab_w96, slab_w32,
            bank_slab, len(cols))


(_BANK_OPS, _SLAB_COL0, _SLAB_W64, _SLAB_W96, _SLAB_W32, _BANK_SLAB,
 NCOLS) = _plan()
NSLABS = len(_SLAB_W64)
F_TOT = NCOLS * TILE_W


def _slab_of(col):
    import bisect
    return bisect.bisect_right(_SLAB_COL0, col) - 1

# Even/odd DFT folding: E[n'] = sum_m wc*Re and O[n'] = sum_m ws*Im
# for n' in [0,512) plus the y[512] column (folded into O's zero column);
# then y[n'] = E+O, y[1024-n'] = E-O.
NE = NLON // 2       # 512
FW = NE + NE + 16    # wc | ws | (y512 col + pad)
CKC = C * KC         # 1024


def build_program():
    from concourse import bacc, bass, masks, mybir, tile

    dt = mybir.dt
    nc = bacc.Bacc("TRN2", target_bir_lowering=False, debug=False,
                   num_devices=NCORES)

    # NOTE: per-slab contiguous DRAM tensors were measured 11us SLOWER
    # than this monolithic strided layout (the engines parallelize
    # better over uniform big-stride partition rows)
    stream = nc.dram_tensor("stream", [128, F_TOT], dt.float16,
                            kind="ExternalInput")
    fmat = nc.dram_tensor("fmat", [128, NBLK * FW + 128], dt.float16,
                          kind="ExternalInput")
    y = nc.dram_tensor("y", [CKC, NLON], dt.float16, kind="ExternalOutput")

    with tile.TileContext(nc) as tc, ExitStack() as ctx:
        sp = ctx.enter_context(
            tc.tile_pool(name="stream", bufs=SP_BUFS))
        cp = ctx.enter_context(tc.tile_pool(name="const", bufs=1))
        fp = ctx.enter_context(tc.tile_pool(name="fsb", bufs=4))
        ysp = ctx.enter_context(tc.tile_pool(name="ysb", bufs=UV_BUFS))
        uvp = ctx.enter_context(tc.tile_pool(name="uv", bufs=UV_BUFS))
        snp = ctx.enter_context(tc.tile_pool(name="snat", bufs=2))
        lhp = ctx.enter_context(tc.tile_pool(name="lhs", bufs=2))
        ps1 = ctx.enter_context(
            tc.tile_pool(name="ps1", bufs=3, space=bass.MemorySpace.PSUM))
        pst = ctx.enter_context(
            tc.tile_pool(name="pst", bufs=1, space=bass.MemorySpace.PSUM))
        ps2 = ctx.enter_context(
            tc.tile_pool(name="ps2", bufs=2, space=bass.MemorySpace.PSUM))

        # fp16 partial accumulator for the blocks handled before the tail:
        # partition = (c2,k64) within strip, free = strip*1024 + n
        acc = cp.tile([128, 8 * NLON], dt.float16)

        slabs = {}

        def get_slab(si, eng=None):
            si = min(si, NSLABS - 1)
            if si not in slabs:
                w = _SLAB_COL0[si + 1] - _SLAB_COL0[si]
                w64 = _SLAB_W64[si]
                st = sp.tile([128, w * TILE_W], dt.float16, tag="slab")
                o0 = _SLAB_COL0[si] * TILE_W
                if eng is None:
                    eng = nc.gpsimd if ALT_DMAQ else nc.sync
                # rows 64:128 go through the sync queue so two queues
                # feed the DMA engines concurrently (K_BSYNC=0 reverts).
                # Trailing slabs' B-pieces ride gpsimd instead: sync's
                # smaller-packet B stream runs time-behind the A queue,
                # and a stage-1 LDW was observed waiting on a trailing
                # slab 9us after the A queue drained
                eng_b = (nc.sync if B_ON_SYNC and si < NSLABS - B_TAIL_GPS
                         else eng)
                w32 = _SLAB_W32[si]
                if A_2RANGE and w32 < w:
                    # columns with K <= 32 skip rows 32:64; the kept
                    # 32:64 piece rides sync so the split slab's pieces
                    # land in parallel across queues
                    eng.dma_start(
                        out=st[0:32, :],
                        in_=stream[0:32, o0:o0 + w * TILE_W])
                    if w32:
                        eng_b.dma_start(
                            out=st[32:64, 0:w32 * TILE_W],
                            in_=stream[32:64, o0:o0 + w32 * TILE_W])
                else:
                    eng.dma_start(
                        out=st[0:64, :],
                        in_=stream[0:64, o0:o0 + w * TILE_W])
                w96 = _SLAB_W96[si]
                if B_3RANGE and w64 and w96 < w64:
                    # columns with K in (64,96] skip rows 96:128 too
                    eng_b.dma_start(
                        out=st[64:96, 0:w64 * TILE_W],
                        in_=stream[64:96, o0:o0 + w64 * TILE_W])
                    if w96:
                        eng_b.dma_start(
                            out=st[96:128, 0:w96 * TILE_W],
                            in_=stream[96:128, o0:o0 + w96 * TILE_W])
                elif w64:
                    eng_b.dma_start(
                        out=st[64:128, 0:w64 * TILE_W],
                        in_=stream[64:128, o0:o0 + w64 * TILE_W])
                slabs[si] = st
            return slabs[si]

        # first slabs from the sync queue (ready before gpsimd's table
        # load); identity comes in with fmat instead of being computed
        # (slab 1 on gpsimd instead measured +5us-or-drift; reverted)
        get_slab(0, nc.sync)
        get_slab(1, nc.sync)
        get_slab(2, nc.sync)

        ident = cp.tile([128, 128], dt.float16)
        nc.sync.dma_start(out=ident[:],
                          in_=fmat[:, NBLK * FW:NBLK * FW + 128])

        deferred = []  # previous block's transpose + DFT work, as thunks

        def emit_dft(tE, oE, tO, oO, lh, fsb, s8, first, last):
            """One block's 3 DFT matmuls for strip s8 into the E bank
            (tile tE cols oE:oE+NE) and O bank (tO cols oO:oO+NE, which
            also takes the y512 column).  Each bank is one accumulation
            group."""
            l0 = lh[:, s8 * 128:s8 * 128 + 128]               # Re rows
            l1 = lh[:, CKC + s8 * 128:CKC + s8 * 128 + 128]   # Im rows
            nc.tensor.matmul(tE[:, oE:oE + NE], l0, fsb[:, 0:NE],
                             start=first, stop=last)
            nc.tensor.matmul(tO[:, oO:oO + NE], l1, fsb[:, NE:2 * NE],
                             start=first, stop=False)
            nc.tensor.matmul(tO[:, oO:oO + 8], l0,
                             fsb[:, 2 * NE:2 * NE + 8],
                             start=False, stop=last)

        def make_deferred(bi, snat_m, fsb, prev):
            """Block's post-stage-1 work: PE transposes into the stage-2
            lhsT layout, then (depending on position in the schedule)
            the DFT matmuls + accumulation.  prev = list of (lhs, fsb)
            for blocks whose DFT is still pending."""
            thunks = []
            lhs = lhp.tile([128, 2 * CKC], dt.float16, tag="lhs")
            lhs_v = lhs[:].rearrange("p (r c k) -> p r c k", r=2, c=C, k=KC)

            def transp_group(g4):
                # 4 channels' PE transposes into one PSUM tile, then a
                # single strided copy into the lhs layout
                pt4 = pst.tile([128, 512], dt.float16, tag="pt")
                for j in range(4):
                    cc = 4 * g4 + j
                    nc.tensor.transpose(pt4[:, 128 * j:128 * (j + 1)],
                                        snat_m[:, :, cc], ident[:])
                # pt4 f = (c4, ri, k) -> lhs f view (ri, c4, k)
                src = pt4[:].rearrange("p (c r k) -> p r c k",
                                       c=4, r=2, k=KC)
                eng = (nc.vector.tensor_copy if g4 % 2 == 0
                       else nc.scalar.copy)
                eng(lhs_v[:, :, 4 * g4:4 * (g4 + 1), :], src)

            for g4 in range(4):
                thunks.append(lambda g4=g4: transp_group(g4))

            pending = prev + [(lhs, fsb)]

            if bi == 0:
                # first processed block: keep lhs, DFT runs paired with
                # the next block
                return thunks, pending

            def dft_strip(s8):
                tail = bi == NBLK - 1
                bb = s8 * NLON
                if tail and s8 % 2 == 1:
                    # odd tail strips take E/O banks from the (now idle)
                    # stage-1 ps1 ring: alternating PSUM pools doubles
                    # the WAR distance per pool, so strips overlap deeper
                    ypE = ps1.tile([128, NE], dt.float32, tag="pb")
                    ypO = ps1.tile([128, NE], dt.float32, tag="pb")
                    yp = None
                    tE, oE, tO, oO = ypE, 0, ypO, 0
                else:
                    yp = ps2.tile([128, NLON], dt.float32, tag="yp")
                    tE, oE, tO, oO = yp, 0, yp, NE
                for j, (lh, fb) in enumerate(pending):
                    emit_dft(tE, oE, tO, oO, lh, fb, s8, first=(j == 0),
                             last=(not tail and j == len(pending) - 1))
                if tail:
                    # append acc (all earlier blocks) into the PSUM
                    # accumulation: yp += I^T @ acc
                    nc.tensor.matmul(tE[:, oE:oE + NE], ident[:],
                                     acc[:, bb:bb + NE],
                                     start=False, stop=True)
                    nc.tensor.matmul(tO[:, oO:oO + NE], ident[:],
                                     acc[:, bb + NE:bb + NLON],
                                     start=False, stop=True)
                a_sl = acc[:, bb:bb + NLON]
                if bi == 1:
                    eng = nc.vector.tensor_copy if s8 % 2 else nc.scalar.copy
                    eng(a_sl, yp[:])
                elif bi == 2:
                    nc.vector.tensor_add(a_sl, a_sl, yp[:])
                else:
                    # tail: the PE folds acc into the same PSUM group via
                    # identity matmuls, so E|O = full spectrum; the
                    # vector engines only evacuate + E/O-fold:
                    #   y[n] = E[n]+O[n], y[1024-n] = E[n]-O[n]
                    uv = uvp.tile([128, NLON], dt.float16, tag="uv")
                    nc.scalar.copy(uv[:, 0:NE], tE[:, oE:oE + NE])
                    nc.vector.tensor_copy(uv[:, NE:NLON],
                                          tO[:, oO:oO + NE])
                    ysb = ysp.tile([128, NLON], dt.float16, tag="ysb")
                    nc.gpsimd.tensor_add(
                        ysb[:, 1:NE], uv[:, 1:NE], uv[:, NE + 1:NLON])
                    nc.vector.tensor_sub(
                        ysb[:, NE + 1:NLON],
                        uv[:, NE - 1:0:-1], uv[:, NLON - 1:NE:-1])
                    nc.scalar.copy(ysb[:, 0:1], uv[:, 0:1])
                    nc.scalar.copy(ysb[:, NE:NE + 1], uv[:, NE:NE + 1])
                    # issue from the (idle) sync queue: an issue on scalar
                    # would make the next strip's ACT copy wait for this
                    # strip's full fold (in-order queue)
                    nc.sync.dma_start(
                        out=y[s8 * 128:(s8 + 1) * 128, :], in_=ysb[:])

            for s8 in range(8):
                thunks.append(lambda s8=s8: dft_strip(s8))
            return thunks, []

        pending = []
        for bi, b in enumerate(BORDER):
            # DFT matrix slice for this block (prefetched during stage 1)
            fsb = fp.tile([128, FW], dt.float16, tag="fsb")
            nc.sync.dma_start(out=fsb[:], in_=fmat[:, b * FW:(b + 1) * FW])

            # S^T staging for this 128-m block:
            #   partition = ri*64+k, free = m_loc*16 + c   (fp16)
            snat = snp.tile([128, 128 * C], dt.float16, tag="snat")
            snat_g = snat[:].rearrange("p (g s two c) -> p g s two c",
                                       g=8, s=8, two=2, c=C)
            snat_m = snat[:].rearrange("p (m c) -> p m c", c=C)

            # ---- stage 1: Legendre matmuls, 8 m-pairs per PSUM bank ----
            for g in range(8):
                G = b * 8 + g
                pos = BANK_SEQ.index(G)
                # issue every slab up to the lookahead horizon
                si_hi = min(_BANK_SLAB[min(pos + PREF_BANKS, NBANK - 1)],
                            NSLABS - 1)
                for si in range(si_hi + 1):
                    get_slab(si)
                # drain deferred units from the previous block FIRST:
                # their inputs are long ready, so the in-order PE fills
                # the wait for this bank's slab with useful work.  Drain
                # nothing at the last two banks: their extractions must
                # not queue behind deferred fold work on DVE/ACT, or the
                # next block's transposes stall on them (measured 2.2us
                # per block transition)
                for _ in range(DRAIN_N if g < DRAIN_CUT else 0):
                    if deferred:
                        deferred.pop(0)()
                pb = ps1.tile([128, 512], dt.float32, tag="pb")
                pb_v = pb[:].rearrange("p (s mj r c) -> p s mj r c",
                                       s=8, mj=2, r=2, c=C)
                ops = _BANK_OPS[G]
                for j, (t, l0, K, col) in enumerate(ops):
                    si_op = _slab_of(col)
                    st = get_slab(si_op)
                    o = (col - _SLAB_COL0[si_op]) * TILE_W
                    s = t % 8
                    if PCT_FP8:
                        lhsT = st[0:K, o:o + PCT_W16].bitcast(dt.float8e3)
                    else:
                        lhsT = st[0:K, o:o + PCT_W16]
                    mov = st[0:K, o + PCT_W16:o + TILE_W]
                    if X_FP8:
                        mov = mov.bitcast(dt.float8e3)
                    nc.tensor.matmul(
                        pb[:, s * 64:(s + 1) * 64],
                        lhsT,
                        mov,
                        start=(j == 0), stop=(j == len(ops) - 1),
                        tile_position=(0, 0))
                # extract diagonal (mi==mj) blocks -> snat (cast fp16),
                # split across DVE and ACT
                for mi in range(2):
                    for r in range(2):
                        eng = (nc.vector.tensor_copy if (mi + r) % 2 == 0
                               else nc.scalar.copy)
                        eng(snat_g[r * 64:(r + 1) * 64, g, :, mi, :],
                            pb_v[mi * 64:(mi + 1) * 64, :, mi, r, :])

            while deferred:
                deferred.pop(0)()
            deferred, pending = make_deferred(bi, snat_m, fsb, pending)

        # last block's work has no next block to hide in
        while deferred:
            deferred.pop(0)()

    nc.compile()
    return nc


def _build_fmat():
    m = np.arange(M_E)
    n2 = np.arange(NE)
    w = np.where(m == 0, 1.0, 2.0)
    ang = 2.0 * np.pi * np.outer(m, n2) / NLON
    wc = (w[:, None] * np.cos(ang)).astype(np.float16)     # E weights
    ws = (-w[:, None] * np.sin(ang)).astype(np.float16)    # O weights
    fz = (w * np.where(m % 2 == 0, 1.0, -1.0)).astype(np.float16)  # y[512]
    fmat = np.zeros((128, NBLK * FW + 128), np.float16)
    for b in range(NBLK):
        sl = slice(b * 128, (b + 1) * 128)
        fmat[:, b * FW:b * FW + NE] = wc[sl]
        fmat[:, b * FW + NE:b * FW + 2 * NE] = ws[sl]
        fmat[:, b * FW + 2 * NE] = fz[sl]
    fmat[:, NBLK * FW:] = np.eye(128, dtype=np.float16)
    return fmat


_ALL_OPS = [op for ops in _BANK_OPS for op in ops]


def _pack_streams(x_re, x_im, pct):
    """Per-core packed stream of shelf-packed (<=128 x TILE_W) tiles.
    Tile layout per row: [pct: PCT_W16 fp16 slots | x: 64 fp16]."""
    import ml_dtypes

    x_re = np.asarray(x_re, np.float32)
    x_im = np.asarray(x_im, np.float32)
    pct = np.asarray(pct, np.float32)

    # x part is core-independent: build once
    template = np.zeros((128, F_TOT), np.float16)
    tv = template.reshape(128, NCOLS, TILE_W)
    for (t, l0, K, col) in _ALL_OPS:
        xr = x_re[0, :, l0:l0 + K, 2 * t:2 * t + 2]   # (c, K, 2m)
        xi = x_im[0, :, l0:l0 + K, 2 * t:2 * t + 2]
        xx = np.stack([xr, xi], axis=0)                # (r, c, K, m)
        xk = xx.transpose(2, 3, 0, 1).reshape(K, 64)
        if X_FP8:
            xk = np.ascontiguousarray(
                xk.astype(ml_dtypes.float8_e3m4)).view(np.float16)
        tv[0:K, col, PCT_W16:] = xk

    streams = []
    for core in range(NCORES):
        k0 = core * KC
        sbuf = template.copy()
        sv = sbuf.reshape(128, NCOLS, TILE_W)
        for (t, l0, K, col) in _ALL_OPS:
            blk = pct[2 * t:2 * t + 2, k0:k0 + KC, l0:l0 + K]  # (2m, 64k, K)
            pk = blk.transpose(2, 0, 1).reshape(K, 128)
            if PCT_FP8:
                pk = np.ascontiguousarray(
                    pk.astype(ml_dtypes.float8_e3m4)).view(np.uint8)
                pk = pk.view(np.float16)               # (K, 64)
            sv[0:K, col, 0:PCT_W16] = pk
        streams.append(sbuf)
    return streams


_NC_CACHE = [None]


def _get_program():
    if _NC_CACHE[0] is None:
        _NC_CACHE[0] = build_program()
    return _NC_CACHE[0]


def run(x_re, x_im, pct, nlon=NLON, trace=False, trace_kwargs=None):
    from concourse.bass_utils import run_bass_kernel_spmd

    assert int(nlon) == NLON
    nc = _get_program()
    fmat = _build_fmat()
    streams = _pack_streams(x_re, x_im, pct)
    in_maps = [{"stream": streams[i], "fmat": fmat} for i in range(NCORES)]
    res = run_bass_kernel_spmd(nc, in_maps, list(range(NCORES)),
                               trace=trace, **(trace_kwargs or {}))
    out = np.empty((B, C, NLAT, NLON), np.float32)
    for core in range(NCORES):
        yc = res.results[core]["y"].astype(np.float32).reshape(C, KC, NLON)
        out[0, :, core * KC:(core + 1) * KC, :] = yc
    return out, res


def kernel(x_re, x_im, pct, nlon=NLON, **_unused):
    out, _ = run(x_re, x_im, pct, nlon)
    return out

